# revision 14
# baseline (speedup 1.0000x reference)
"""Trainium2 Bass kernel for a single-head causal transformer block.

B=4, T=2048, D=1024, MLP 4x. 8 NeuronCores, SPMD: core c handles batch c//2,
token half c%2 (1024 own tokens). Each core recomputes k/v for its batch's
full 2048 tokens (cheaper than a collective). All per-core divergence (token
order, causal mask, q indices) is data-driven so every core runs the same NEFF.

Matmuls run in float32r (TF32-like, full PE rate at free-dim>=256); the
softmax weights and v go through bf16 (post-LN2 renormalization keeps the
effect tiny). LN gains/biases are folded into the next matmul host-side.
"""
import sys
sys.path.insert(0, '/opt/trn_rl_repo')

import math
import numpy as np

import concourse.bass as bass
import concourse.tile as tile
from concourse import mybir
from concourse.alu_op_type import AluOpType

B, T, D = 4, 2048, 1024
DM = 4 * D          # mlp hidden
OWN = 1024          # tokens per core
EPS = 1e-5
SCALE = 1.0 / math.sqrt(T)
F32 = mybir.dt.float32
F32R = mybir.dt.float32r
BF16 = mybir.dt.float16  # fp16: f32r-grade mantissa for attention path
AF = mybir.ActivationFunctionType
N_CORES = 8


def _split_waits(nc, max_waits=1):
    """This walrus build allows at most ONE sync-wait per instruction.
    Move extra waits onto same-engine NoOps placed just before."""
    n_added = 0
    for f in nc.m.functions:
        for blk in f.blocks:
            out, dirty = [], False
            for inst in blk.instructions:
                si = inst.sync_info
                if si is not None and len(si.on_wait) > max_waits:
                    waits = list(si.on_wait)
                    extra, keep = waits[:-max_waits], waits[-max_waits:]
                    for k, w in enumerate(extra):
                        out.append(mybir.InstNoOp(
                            name=f"{inst.name}-ws{k}",
                            engine=inst.engine,
                            sync_info=mybir.SyncInfo(on_wait=[w], on_update=[]),
                            bass_nofuse=True,
                        ))
                        n_added += 1
                    inst.sync_info = mybir.SyncInfo(
                        on_wait=keep, on_update=list(si.on_update))
                    dirty = True
                out.append(inst)
            if dirty:
                blk.instructions = out
    return n_added


def build_nc(rep=1, trace_sim=False, phases='ABCDEFGHILW'):
    from concourse.masks import make_identity

    nc = bass.Bass("TRN2", target_bir_lowering=False, debug=False)
    d = lambda name, shape, dt=F32, out=False: nc.dram_tensor(
        name, shape, dt, kind="ExternalOutput" if out else "ExternalInput").ap()

    xb = d("xb", [T, D])                 # reordered tokens (own first)
    wk, wq, wv, wp = (d(n, [D, D]) for n in ("wk", "wq", "wv", "wp"))
    w1 = d("w1", [D, DM])
    w2 = d("w2", [DM, D])
    bkT = d("bkT", [128, 8])             # k-bias, [dout%128, dout//128]
    bqT = d("bqT", [128, 8])
    bv = d("bv", [1, D])
    bp = d("bp", [1, D])
    b1T = d("b1T", [128, 32])
    b2 = d("b2", [1, D])
    qidx = d("qidx", [128, OWN])         # global q index per own-token column
    kidx = d("kidx", [128, 16])          # global kv index per (partition, kblock)
    out = d("out", [OWN, D], out=True)
    v_spill = nc.dram_tensor("v_spill", [T, D], BF16).ap()
    av_spill = nc.dram_tensor("av_spill", [D, OWN], F32R).ap()
    rc_spill = nc.dram_tensor("rc_spill", [128, 8], F32).ap()

    # [ (s p) n -> p s n ] views: row r = s*128+p
    wk3 = wk.rearrange("(s p) n -> p s n", p=128)
    wq3 = wq.rearrange("(s p) n -> p s n", p=128)
    wv3 = wv.rearrange("(s p) n -> p s n", p=128)
    wp3 = wp.rearrange("(s p) n -> p s n", p=128)
    w13 = w1.rearrange("(s p) n -> p s n", p=128)
    w23 = w2.rearrange("(s p) n -> p s n", p=128)
    vs3 = v_spill.rearrange("(s p) n -> p s n", p=128)
    av3 = av_spill.rearrange("(s p) n -> p s n", p=128)

    for r in range(rep):
        # ================= TC1: LN1, q/k/v, attention =================
        with tile.TileContext(nc, pool_alloc_mode="queue", trace_sim=trace_sim) as tc:
            cst = tc.alloc_tile_pool(name="cst", bufs=1)
            ident = cst.tile([128, 128], F32)
            make_identity(nc, ident[:])
            bkT_s = cst.tile([128, 8], F32)
            nc.sync.dma_start(bkT_s[:], bkT)
            bqT_s = cst.tile([128, 8], F32)
            nc.sync.dma_start(bqT_s[:], bqT)
            qidx_s = cst.tile([128, OWN], F32)
            nc.sync.dma_start(qidx_s[:], qidx)
            kidx_s = cst.tile([128, 16], F32)
            nc.sync.dma_start(kidx_s[:], kidx)
            bv_s = cst.tile([128, D], F32)
            nc.sync.dma_start(bv_s[:], bv.broadcast_to((128, D)))
            ones_f = cst.tile([128, 1], F32)
            nc.vector.memset(ones_f[:], 1.0)
            ones_b = cst.tile([128, 1], BF16)
            nc.vector.tensor_copy(ones_b[:], ones_f[:])
            eps_s = cst.tile([128, 1], F32)
            nc.vector.memset(eps_s[:], EPS)
            sc_s = cst.tile([128, 1], F32)
            nc.vector.memset(sc_s[:], SCALE)
            recip = cst.tile([128, 8], F32)

            ps_pool = tc.alloc_tile_pool(name="ps", bufs=6, space="PSUM")
            pst_pool = tc.alloc_tile_pool(name="pst", bufs=2, space="PSUM")

            def ln_stats(pool, src, tag):
                st = pool.tile([128, 2, 6], F32, tag=f"{tag}_st")
                sv = src.rearrange("p (s d) -> p s d", s=2)
                nc.vector.bn_stats(st[:, 0, :], sv[:, 0, :])
                nc.vector.bn_stats(st[:, 1, :], sv[:, 1, :])
                mv = pool.tile([128, 2], F32, tag=f"{tag}_mv")
                nc.vector.bn_aggr(mv[:], st[:])
                rstd = pool.tile([128, 1], F32, tag=f"{tag}_rs")
                nc.scalar.activation(rstd[:], mv[:, 1:2], AF.Sqrt, bias=eps_s[:, 0:1])
                nc.vector.reciprocal(rstd[:], rstd[:])
                return mv, rstd

            kT_pool = tc.alloc_tile_pool(name="kT", bufs=1)
            kT = kT_pool.tile([128, 8, T], F32R)
            qT_pool = tc.alloc_tile_pool(name="qT", bufs=1)
            qT = qT_pool.tile([128, 8, OWN], F32R)

            # ---- merged LN1 + k/q/v projections, per 1024-token group ----
            work = tc.alloc_tile_pool(name="work", bufs=3)
            hTg_pool = tc.alloc_tile_pool(name="hTg", bufs=1)
            for grp in range(2):
                hTg = hTg_pool.tile([128, 8, 1024], F32R, tag="hTg")
                for tbl in range(8):
                    x_t = work.tile([128, D], F32, tag="x", bufs=2)
                    nc.sync.dma_start(
                        x_t[:], xb[(grp * 8 + tbl) * 128:(grp * 8 + tbl + 1) * 128, :])
                    mv, rstd = ln_stats(work, x_t[:], "a")
                    h_t = work.tile([128, D], F32, tag="h", bufs=2)
                    nc.vector.tensor_scalar(h_t[:], x_t[:], mv[:, 0:1], rstd[:],
                                            op0=AluOpType.subtract, op1=AluOpType.mult)
                    for db in range(8):
                        pt = pst_pool.tile([128, 128], F32, tag="pst")
                        nc.tensor.transpose(pt[:], h_t[:, db * 128:(db + 1) * 128],
                                            ident[:])
                        eng = nc.scalar.copy if (db + tbl) % 2 else nc.vector.tensor_copy
                        eng(hTg[:, db, tbl * 128:(tbl + 1) * 128], pt[:])
                # kT columns for this group
                for dob in range(8):
                    wt = work.tile([128, 8, 128], F32R, tag="wk", bufs=2)
                    nc.sync.dma_start(
                        wt[:], wk3[:, :, dob * 128:(dob + 1) * 128].bitcast(F32R))
                    for ch in range(2):
                        ps = ps_pool.tile([128, 512], F32, tag="ps")
                        for dib in range(8):
                            nc.tensor.matmul(ps[:], lhsT=wt[:, dib, :],
                                             rhs=hTg[:, dib, ch * 512:(ch + 1) * 512],
                                             start=(dib == 0), stop=(dib == 7))
                        o = kT[:, dob, grp * 1024 + ch * 512:grp * 1024 + (ch + 1) * 512]
                        if ch % 2:
                            nc.scalar.activation(o, ps[:], AF.Identity,
                                                 bias=bkT_s[:, dob:dob + 1])
                        else:
                            nc.vector.tensor_scalar_add(o, ps[:], bkT_s[:, dob:dob + 1])
                # qT (own tokens are always group 0 — same on every core)
                if grp == 0:
                    for dob in range(8):
                        wt = work.tile([128, 8, 128], F32R, tag="wk", bufs=2)
                        nc.sync.dma_start(
                            wt[:], wq3[:, :, dob * 128:(dob + 1) * 128].bitcast(F32R))
                        for ch in range(2):
                            ps = ps_pool.tile([128, 512], F32, tag="ps")
                            for dib in range(8):
                                nc.tensor.matmul(ps[:], lhsT=wt[:, dib, :],
                                                 rhs=hTg[:, dib, ch * 512:(ch + 1) * 512],
                                                 start=(dib == 0), stop=(dib == 7))
                            o = qT[:, dob, ch * 512:(ch + 1) * 512]
                            if ch % 2:
                                nc.scalar.activation(o, ps[:], AF.Identity,
                                                     bias=bqT_s[:, dob:dob + 1])
                            else:
                                nc.vector.tensor_scalar_add(o, ps[:],
                                                            bqT_s[:, dob:dob + 1])
                # v rows for this group -> bf16 spill
                for ch in range(2):
                    wt = work.tile([128, 8, 512], F32R, tag="wv", bufs=1)
                    nc.sync.dma_start(
                        wt[:], wv3[:, :, ch * 512:(ch + 1) * 512].bitcast(F32R))
                    for tbl in range(8):
                        ps = ps_pool.tile([128, 512], F32, tag="ps")
                        for dib in range(8):
                            nc.tensor.matmul(
                                ps[:], lhsT=hTg[:, dib, tbl * 128:(tbl + 1) * 128],
                                rhs=wt[:, dib, :], start=(dib == 0), stop=(dib == 7))
                        vt = work.tile([128, 512], BF16, tag="vt")
                        nc.vector.tensor_tensor(vt[:], ps[:],
                                                bv_s[:, ch * 512:(ch + 1) * 512],
                                                op=AluOpType.add)
                        tb = grp * 8 + tbl
                        nc.sync.dma_start(
                            v_spill[tb * 128:(tb + 1) * 128, ch * 512:(ch + 1) * 512],
                            vt[:])
            hTg_pool.release()
            work.release()

            # ---- avT allocated before eT (it outlives eT; LIFO) ----
            avT_pool = tc.alloc_tile_pool(name="avT", bufs=1)
            avT = avT_pool.tile([128, 8, OWN], F32R)

            # ---- E: eT = mask * exp(scale * kT^T q), bf16 [k, q-own] ----
            eT_pool = tc.alloc_tile_pool(name="eT", bufs=1)
            eT = eT_pool.tile([128, 16, OWN], BF16)
            tmpE = tc.alloc_tile_pool(name="tmpE", bufs=3)
            for ch in range(2 if 'E' in phases else 0):
                for kb in range(16):
                    ps = ps_pool.tile([128, 512], F32, tag="ps")
                    for dib in range(8):
                        nc.tensor.matmul(ps[:], lhsT=kT[:, dib, kb * 128:(kb + 1) * 128],
                                         rhs=qT[:, dib, ch * 512:(ch + 1) * 512],
                                         start=(dib == 0), stop=(dib == 7))
                    tmp = tmpE.tile([128, 512], F32, tag="t")
                    nc.scalar.activation(tmp[:], ps[:], AF.Exp, scale=sc_s[:, 0:1])
                    nc.vector.scalar_tensor_tensor(
                        eT[:, kb, ch * 512:(ch + 1) * 512],
                        in0=qidx_s[:, ch * 512:(ch + 1) * 512],
                        scalar=kidx_s[:, kb:kb + 1],
                        in1=tmp[:],
                        op0=AluOpType.is_ge, op1=AluOpType.mult)
            tmpE.release()

            # ---- F: avT = v^T eT ; rowsums -> recip ; spill avT/recip ----
            vin_pool = tc.alloc_tile_pool(name="vin", bufs=1)
            v_sb = vin_pool.tile([128, 16, D], BF16)
            for kb in range(16 if 'F' in phases else 0):
                nc.sync.dma_start(v_sb[:, kb, :], vs3[:, kb, :])
            for ch in range(2 if 'F' in phases else 0):
                for dob in range(8):
                    ps = ps_pool.tile([128, 512], F32, tag="ps")
                    for kb in range(16):
                        nc.tensor.matmul(ps[:],
                                         lhsT=v_sb[:, kb, dob * 128:(dob + 1) * 128],
                                         rhs=eT[:, kb, ch * 512:(ch + 1) * 512],
                                         start=(kb == 0), stop=(kb == 15))
                    o = avT[:, dob, ch * 512:(ch + 1) * 512]
                    if dob % 2:
                        nc.scalar.copy(o, ps[:])
                    else:
                        nc.vector.tensor_copy(o, ps[:])
            for qb in range(8 if 'F' in phases else 0):
                ps = ps_pool.tile([128, 512], F32, tag="ps")
                for kb in range(16):
                    nc.tensor.matmul(ps[:, 0:1], lhsT=eT[:, kb, qb * 128:(qb + 1) * 128],
                                     rhs=ones_b[:, 0:1],
                                     start=(kb == 0), stop=(kb == 15))
                nc.vector.reciprocal(recip[:, qb:qb + 1], ps[:, 0:1])
            for dob in range(8 if 'F' in phases else 0):
                nc.sync.dma_start(av3[:, dob, :], avT[:, dob, :])
            if 'F' in phases:
                nc.sync.dma_start(rc_spill, recip[:])
            else:
                nc.vector.memset(recip[:], 1.0)
                nc.sync.dma_start(rc_spill, recip[:])
            vin_pool.release()
            eT_pool.release()
            avT_pool.release()
            qT_pool.release()
            kT_pool.release()
            pst_pool.release()
            ps_pool.release()
            cst.release()

        # ================= TC2: proj + LN2 + MLP =================
        with tile.TileContext(nc, pool_alloc_mode="queue", trace_sim=trace_sim) as tc:
            cst = tc.alloc_tile_pool(name="cst2", bufs=1)
            ident = cst.tile([128, 128], F32)
            make_identity(nc, ident[:])
            bp_s = cst.tile([128, D], F32)
            nc.sync.dma_start(bp_s[:], bp.broadcast_to((128, D)))
            b2_s = cst.tile([128, D], F32)
            nc.sync.dma_start(b2_s[:], b2.broadcast_to((128, D)))
            b1T_s = cst.tile([128, 32], F32)
            nc.sync.dma_start(b1T_s[:], b1T)
            eps_s = cst.tile([128, 1], F32)
            nc.vector.memset(eps_s[:], EPS)
            recip = cst.tile([128, 8], F32)
            nc.sync.dma_start(recip[:], rc_spill)

            ps_pool = tc.alloc_tile_pool(name="ps2", bufs=6, space="PSUM")
            pst_pool = tc.alloc_tile_pool(name="pst2", bufs=2, space="PSUM")

            def ln_stats2(pool, src, tag):
                st = pool.tile([128, 2, 6], F32, tag=f"{tag}_st")
                sv = src.rearrange("p (s d) -> p s d", s=2)
                nc.vector.bn_stats(st[:, 0, :], sv[:, 0, :])
                nc.vector.bn_stats(st[:, 1, :], sv[:, 1, :])
                mv = pool.tile([128, 2], F32, tag=f"{tag}_mv")
                nc.vector.bn_aggr(mv[:], st[:])
                rstd = pool.tile([128, 1], F32, tag=f"{tag}_rs")
                nc.scalar.activation(rstd[:], mv[:, 1:2], AF.Sqrt, bias=eps_s[:, 0:1])
                nc.vector.reciprocal(rstd[:], rstd[:])
                return mv, rstd

            # h2T before avT2: h2T outlives it (LIFO)
            h2T_pool = tc.alloc_tile_pool(name="h2T", bufs=1)
            h2T = h2T_pool.tile([128, 8, OWN], F32R)
            avT_pool = tc.alloc_tile_pool(name="avT2", bufs=1)
            avT = avT_pool.tile([128, 8, OWN], F32R)
            for dob in range(8):
                nc.sync.dma_start(avT[:, dob, :], av3[:, dob, :])

            # ---- G: a = recip * (avT^T Wp) + bp ; LN2 ; transpose ----
            wp_pool = tc.alloc_tile_pool(name="wp", bufs=1)
            wpt = wp_pool.tile([128, 8, D], F32R)
            nc.sync.dma_start(wpt[:], wp3.bitcast(F32R))
            ga = tc.alloc_tile_pool(name="ga", bufs=3)
            for tb in range(8 if 'G' in phases else 0):
                a_t = ga.tile([128, D], F32, tag="a")
                for ch in range(2):
                    ps = ps_pool.tile([128, 512], F32, tag="ps")
                    for dob in range(8):
                        nc.tensor.matmul(ps[:], lhsT=avT[:, dob, tb * 128:(tb + 1) * 128],
                                         rhs=wpt[:, dob, ch * 512:(ch + 1) * 512],
                                         start=(dob == 0), stop=(dob == 7))
                    nc.vector.scalar_tensor_tensor(
                        a_t[:, ch * 512:(ch + 1) * 512], in0=ps[:],
                        scalar=recip[:, tb:tb + 1],
                        in1=bp_s[:, ch * 512:(ch + 1) * 512],
                        op0=AluOpType.mult, op1=AluOpType.add)
                if 'L' not in phases:
                    continue
                mv, rstd = ln_stats2(ga, a_t[:], "g")
                h2_t = ga.tile([128, D], F32, tag="h2")
                nc.vector.tensor_scalar(h2_t[:], a_t[:], mv[:, 0:1], rstd[:],
                                        op0=AluOpType.subtract, op1=AluOpType.mult)
                for db in range(8):
                    pt = pst_pool.tile([128, 128], F32, tag="pst")
                    nc.tensor.transpose(pt[:], h2_t[:, db * 128:(db + 1) * 128], ident[:])
                    eng = nc.scalar.copy if (db + tb) % 2 else nc.vector.tensor_copy
                    eng(h2T[:, db, tb * 128:(tb + 1) * 128], pt[:])
            ga.release()
            wp_pool.release()
            avT_pool.release()

            # ---- H: m1T = gelu(W1^T h2 + b1) [m, tok] ----
            m1_pool = tc.alloc_tile_pool(name="m1", bufs=1)
            m1T = m1_pool.tile([128, 32, OWN], F32R)
            w1s = tc.alloc_tile_pool(name="w1s", bufs=3)
            wt = None
            for mb in range(32 if 'H' in phases else 0):
                if 'W' in phases or mb == 0:
                    wt = w1s.tile([128, 8, 128], F32R, tag="w1")
                    nc.sync.dma_start(wt[:], w13[:, :, mb * 128:(mb + 1) * 128].bitcast(F32R))
                for ch in range(2):
                    ps = ps_pool.tile([128, 512], F32, tag="ps")
                    for dib in range(8):
                        nc.tensor.matmul(ps[:], lhsT=wt[:, dib, :],
                                         rhs=h2T[:, dib, ch * 512:(ch + 1) * 512],
                                         start=(dib == 0), stop=(dib == 7))
                    nc.scalar.activation(m1T[:, mb, ch * 512:(ch + 1) * 512], ps[:],
                                         AF.Gelu_apprx_tanh, bias=b1T_s[:, mb:mb + 1])
            w1s.release()

            # ---- I: out = m1 @ W2 + b2 ----
            w2s = tc.alloc_tile_pool(name="w2s", bufs=1)
            ost = tc.alloc_tile_pool(name="ost", bufs=3)
            wt = None
            for c4 in range(4 if 'I' in phases else 0):
                if 'W' in phases or c4 == 0:
                    wt = w2s.tile([128, 32, 256], F32R, tag="w2")
                    nc.sync.dma_start(wt[:], w23[:, :, c4 * 256:(c4 + 1) * 256].bitcast(F32R))
                for tb in range(8):
                    ps = ps_pool.tile([128, 512], F32, tag="ps")
                    for mb in range(32):
                        nc.tensor.matmul(ps[:, 0:256],
                                         lhsT=m1T[:, mb, tb * 128:(tb + 1) * 128],
                                         rhs=wt[:, mb, :],
                                         start=(mb == 0), stop=(mb == 31))
                    ot = ost.tile([128, 256], F32, tag="o")
                    nc.vector.tensor_tensor(ot[:], ps[:, 0:256],
                                            b2_s[:, c4 * 256:(c4 + 1) * 256],
                                            op=AluOpType.add)
                    nc.sync.dma_start(
                        out[tb * 128:(tb + 1) * 128, c4 * 256:(c4 + 1) * 256], ot[:])
            if 'I' not in phases:
                for tb in range(8):
                    ot = ost.tile([128, D], F32, tag="o2")
                    nc.vector.memset(ot[:], 0.0)
                    nc.sync.dma_start(out[tb * 128:(tb + 1) * 128, :], ot[:])
            ost.release()
            w2s.release()
            m1_pool.release()
            h2T_pool.release()
            pst_pool.release()
            ps_pool.release()
            cst.release()

    _split_waits(nc)
    return nc


def make_in_maps(x, ln1_g, ln1_b, Wqkv, bqkv, Wproj, bproj, ln2_g, ln2_b, W1, b1, W2, b2):
    f = np.float32
    x = np.asarray(x, f)
    Wqkv = np.asarray(Wqkv, f)
    bqkv = np.asarray(bqkv, f)
    g1 = np.asarray(ln1_g, f)
    be1 = np.asarray(ln1_b, f)
    g2 = np.asarray(ln2_g, f)
    be2 = np.asarray(ln2_b, f)
    Wk_r, Wq_r, Wv_r = Wqkv[:, :D], Wqkv[:, D:2 * D], Wqkv[:, 2 * D:]
    wkf = np.ascontiguousarray(g1[:, None] * Wk_r)
    wqf = np.ascontiguousarray(g1[:, None] * Wq_r)
    wvf = np.ascontiguousarray(g1[:, None] * Wv_r)
    bk = be1 @ Wk_r + bqkv[:D]
    bq = be1 @ Wq_r + bqkv[D:2 * D]
    bvv = be1 @ Wv_r + bqkv[2 * D:]
    W1 = np.asarray(W1, f)
    w1f = np.ascontiguousarray(g2[:, None] * W1)
    b1f = be2 @ W1 + np.asarray(b1, f)
    common = {
        "wk": wkf, "wq": wqf, "wv": wvf,
        "wp": np.ascontiguousarray(np.asarray(Wproj, f)),
        "w1": w1f, "w2": np.ascontiguousarray(np.asarray(W2, f)),
        "bkT": np.ascontiguousarray(bk.reshape(8, 128).T.astype(f)),
        "bqT": np.ascontiguousarray(bq.reshape(8, 128).T.astype(f)),
        "bv": bvv.reshape(1, D).astype(f),
        "bp": np.asarray(bproj, f).reshape(1, D),
        "b1T": np.ascontiguousarray(b1f.reshape(32, 128).T.astype(f)),
        "b2": np.asarray(b2, f).reshape(1, D),
    }
    in_maps = []
    for c in range(N_CORES):
        b, half = c // 2, c % 2
        if half == 0:
            perm = np.arange(T)
        else:
            perm = np.concatenate([np.arange(OWN, T), np.arange(OWN)])
        m = dict(common)
        m["xb"] = np.ascontiguousarray(x[b][perm])
        m["qidx"] = np.broadcast_to(
            np.arange(half * OWN, half * OWN + OWN, dtype=f), (128, OWN)).copy()
        m["kidx"] = np.ascontiguousarray(perm.reshape(16, 128).T.astype(f))
        in_maps.append(m)
    return in_maps


_CACHE = {}


def kernel(**inputs) -> np.ndarray:
    from concourse.bass_utils import run_bass_kernel_spmd
    if "nc" not in _CACHE:
        _CACHE["nc"] = build_nc(rep=1)
    nc = _CACHE["nc"]
    in_maps = make_in_maps(**inputs)
    res = run_bass_kernel_spmd(nc, in_maps, core_ids=list(range(N_CORES)))
    full = np.empty((B, T, D), np.float32)
    for c in range(N_CORES):
        b, half = c // 2, c % 2
        full[b, half * OWN:(half + 1) * OWN, :] = res.results[c]["out"]
    return full


# revision 15
# speedup vs baseline: 2.6923x; 2.6923x over previous
"""Trainium2 Bass kernel for a single-head causal transformer block.

B=4, T=2048, D=1024, MLP 4x. 8 NeuronCores, SPMD: core c handles batch c//2,
token half c%2 (1024 own tokens). Each core recomputes k/v for its batch's
full 2048 tokens (cheaper than a collective). All per-core divergence (token
order, causal mask, q indices) is data-driven so every core runs the same NEFF.

Matmuls run in float32r (TF32-like, full PE rate at free-dim>=256); the
softmax weights and v go through bf16 (post-LN2 renormalization keeps the
effect tiny). LN gains/biases are folded into the next matmul host-side.
"""
import sys
sys.path.insert(0, '/opt/trn_rl_repo')

import math
import numpy as np

import concourse.bass as bass
import concourse.tile as tile
from concourse import mybir
from concourse.alu_op_type import AluOpType

B, T, D = 4, 2048, 1024
DM = 4 * D          # mlp hidden
OWN = 1024          # tokens per core
EPS = 1e-5
SCALE = 1.0 / math.sqrt(T)
F32 = mybir.dt.float32
F32R = mybir.dt.float32r
BF16 = mybir.dt.float16  # fp16: f32r-grade mantissa for attention path
AF = mybir.ActivationFunctionType
N_CORES = 8


def _split_waits(nc, max_waits=1):
    """This walrus build allows at most ONE sync-wait per instruction.
    Move extra waits onto same-engine NoOps placed just before."""
    n_added = 0
    for f in nc.m.functions:
        for blk in f.blocks:
            out, dirty = [], False
            for inst in blk.instructions:
                si = inst.sync_info
                if si is not None and len(si.on_wait) > max_waits:
                    waits = list(si.on_wait)
                    extra, keep = waits[:-max_waits], waits[-max_waits:]
                    for k, w in enumerate(extra):
                        out.append(mybir.InstNoOp(
                            name=f"{inst.name}-ws{k}",
                            engine=inst.engine,
                            sync_info=mybir.SyncInfo(on_wait=[w], on_update=[]),
                            bass_nofuse=True,
                        ))
                        n_added += 1
                    inst.sync_info = mybir.SyncInfo(
                        on_wait=keep, on_update=list(si.on_update))
                    dirty = True
                out.append(inst)
            if dirty:
                blk.instructions = out
    return n_added


def build_nc(rep=1, trace_sim=False, phases='ABCDEFGHILW'):
    from concourse.masks import make_identity

    nc = bass.Bass("TRN2", target_bir_lowering=False, debug=False)
    d = lambda name, shape, dt=F32, out=False: nc.dram_tensor(
        name, shape, dt, kind="ExternalOutput" if out else "ExternalInput").ap()

    xb = d("xb", [T, D])                 # reordered tokens (own first)
    wk, wq, wv, wp = (d(n, [D, D]) for n in ("wk", "wq", "wv", "wp"))
    w1 = d("w1", [D, DM])
    w2 = d("w2", [DM, D])
    bkT = d("bkT", [128, 8])             # k-bias, [dout%128, dout//128]
    bqT = d("bqT", [128, 8])
    bv = d("bv", [1, D])
    bp = d("bp", [1, D])
    b1T = d("b1T", [128, 32])
    b2 = d("b2", [1, D])
    qidx = d("qidx", [128, OWN])         # global q index per own-token column
    kidx = d("kidx", [128, 16])          # global kv index per (partition, kblock)
    out = d("out", [OWN, D], out=True)
    v_spill = nc.dram_tensor("v_spill", [T, D], BF16).ap()
    av_spill = nc.dram_tensor("av_spill", [D, OWN], F32R).ap()
    rc_spill = nc.dram_tensor("rc_spill", [128, 8], F32).ap()

    # [ (s p) n -> p s n ] views: row r = s*128+p
    wk3 = wk.rearrange("(s p) n -> p s n", p=128)
    wq3 = wq.rearrange("(s p) n -> p s n", p=128)
    wv3 = wv.rearrange("(s p) n -> p s n", p=128)
    wp3 = wp.rearrange("(s p) n -> p s n", p=128)
    w13 = w1.rearrange("(s p) n -> p s n", p=128)
    w23 = w2.rearrange("(s p) n -> p s n", p=128)
    vs3 = v_spill.rearrange("(s p) n -> p s n", p=128)
    av3 = av_spill.rearrange("(s p) n -> p s n", p=128)

    for r in range(rep):
        # ================= TC1: LN1, q/k/v, attention =================
        with tile.TileContext(nc, pool_alloc_mode="queue", trace_sim=trace_sim) as tc:
            cst = tc.alloc_tile_pool(name="cst", bufs=1)
            ident = cst.tile([128, 128], F32)
            make_identity(nc, ident[:])
            bkT_s = cst.tile([128, 8], F32)
            nc.sync.dma_start(bkT_s[:], bkT)
            bqT_s = cst.tile([128, 8], F32)
            nc.sync.dma_start(bqT_s[:], bqT)
            qidx_s = cst.tile([128, OWN], F32)
            nc.sync.dma_start(qidx_s[:], qidx)
            kidx_s = cst.tile([128, 16], F32)
            nc.sync.dma_start(kidx_s[:], kidx)
            bv_s = cst.tile([128, D], F32)
            nc.sync.dma_start(bv_s[:], bv.broadcast_to((128, D)))
            ones_f = cst.tile([128, 1], F32)
            nc.vector.memset(ones_f[:], 1.0)
            ones_b = cst.tile([128, 1], BF16)
            nc.vector.tensor_copy(ones_b[:], ones_f[:])
            eps_s = cst.tile([128, 1], F32)
            nc.vector.memset(eps_s[:], EPS)
            sc_s = cst.tile([128, 1], F32)
            nc.vector.memset(sc_s[:], SCALE)
            recip = cst.tile([128, 8], F32)

            ps_pool = tc.alloc_tile_pool(name="ps", bufs=6, space="PSUM")
            pst_pool = tc.alloc_tile_pool(name="pst", bufs=2, space="PSUM")

            def ln_stats(pool, src, tag):
                st = pool.tile([128, 2, 6], F32, tag=f"{tag}_st")
                sv = src.rearrange("p (s d) -> p s d", s=2)
                nc.vector.bn_stats(st[:, 0, :], sv[:, 0, :])
                nc.vector.bn_stats(st[:, 1, :], sv[:, 1, :])
                mv = pool.tile([128, 2], F32, tag=f"{tag}_mv")
                nc.vector.bn_aggr(mv[:], st[:])
                rstd = pool.tile([128, 1], F32, tag=f"{tag}_rs")
                nc.scalar.activation(rstd[:], mv[:, 1:2], AF.Sqrt, bias=eps_s[:, 0:1])
                nc.vector.reciprocal(rstd[:], rstd[:])
                return mv, rstd

            kT_pool = tc.alloc_tile_pool(name="kT", bufs=1)
            kT = kT_pool.tile([128, 8, T], F32R)
            qT_pool = tc.alloc_tile_pool(name="qT", bufs=1)
            qT = qT_pool.tile([128, 8, OWN], F32R)

            # ---- merged LN1 + k/q/v projections, per 1024-token group ----
            work = tc.alloc_tile_pool(name="work", bufs=3)
            hTg_pool = tc.alloc_tile_pool(name="hTg", bufs=1)
            for grp in range(2):
                hTg = hTg_pool.tile([128, 8, 1024], F32R, tag="hTg")
                for tbl in range(8):
                    x_t = work.tile([128, D], F32, tag="x", bufs=2)
                    nc.sync.dma_start(
                        x_t[:], xb[(grp * 8 + tbl) * 128:(grp * 8 + tbl + 1) * 128, :])
                    mv, rstd = ln_stats(work, x_t[:], "a")
                    h_t = work.tile([128, D], F32, tag="h", bufs=2)
                    nc.vector.tensor_scalar(h_t[:], x_t[:], mv[:, 0:1], rstd[:],
                                            op0=AluOpType.subtract, op1=AluOpType.mult)
                    for db in range(8):
                        pt = pst_pool.tile([128, 128], F32, tag="pst")
                        nc.tensor.transpose(pt[:], h_t[:, db * 128:(db + 1) * 128],
                                            ident[:])
                        nc.vector.tensor_copy(hTg[:, db, tbl * 128:(tbl + 1) * 128], pt[:])
                # kT columns for this group
                for dob in range(8):
                    wt = work.tile([128, 8, 128], F32R, tag="wk", bufs=2)
                    nc.sync.dma_start(
                        wt[:], wk3[:, :, dob * 128:(dob + 1) * 128].bitcast(F32R))
                    for ch in range(2):
                        ps = ps_pool.tile([128, 512], F32, tag="ps")
                        for dib in range(8):
                            nc.tensor.matmul(ps[:], lhsT=wt[:, dib, :],
                                             rhs=hTg[:, dib, ch * 512:(ch + 1) * 512],
                                             start=(dib == 0), stop=(dib == 7))
                        o = kT[:, dob, grp * 1024 + ch * 512:grp * 1024 + (ch + 1) * 512]
                        if ch % 2:
                            nc.scalar.activation(o, ps[:], AF.Identity,
                                                 bias=bkT_s[:, dob:dob + 1])
                        else:
                            nc.vector.tensor_scalar_add(o, ps[:], bkT_s[:, dob:dob + 1])
                # qT (own tokens are always group 0 — same on every core)
                if grp == 0:
                    for dob in range(8):
                        wt = work.tile([128, 8, 128], F32R, tag="wk", bufs=2)
                        nc.sync.dma_start(
                            wt[:], wq3[:, :, dob * 128:(dob + 1) * 128].bitcast(F32R))
                        for ch in range(2):
                            ps = ps_pool.tile([128, 512], F32, tag="ps")
                            for dib in range(8):
                                nc.tensor.matmul(ps[:], lhsT=wt[:, dib, :],
                                                 rhs=hTg[:, dib, ch * 512:(ch + 1) * 512],
                                                 start=(dib == 0), stop=(dib == 7))
                            o = qT[:, dob, ch * 512:(ch + 1) * 512]
                            if ch % 2:
                                nc.scalar.activation(o, ps[:], AF.Identity,
                                                     bias=bqT_s[:, dob:dob + 1])
                            else:
                                nc.vector.tensor_scalar_add(o, ps[:],
                                                            bqT_s[:, dob:dob + 1])
                # v rows for this group -> bf16 spill
                for ch in range(2):
                    wt = work.tile([128, 8, 512], F32R, tag="wv", bufs=1)
                    nc.sync.dma_start(
                        wt[:], wv3[:, :, ch * 512:(ch + 1) * 512].bitcast(F32R))
                    for tbl in range(8):
                        ps = ps_pool.tile([128, 512], F32, tag="ps")
                        for dib in range(8):
                            nc.tensor.matmul(
                                ps[:], lhsT=hTg[:, dib, tbl * 128:(tbl + 1) * 128],
                                rhs=wt[:, dib, :], start=(dib == 0), stop=(dib == 7))
                        vt = work.tile([128, 512], BF16, tag="vt")
                        nc.vector.tensor_tensor(vt[:], ps[:],
                                                bv_s[:, ch * 512:(ch + 1) * 512],
                                                op=AluOpType.add)
                        tb = grp * 8 + tbl
                        nc.sync.dma_start(
                            v_spill[tb * 128:(tb + 1) * 128, ch * 512:(ch + 1) * 512],
                            vt[:])
            hTg_pool.release()
            work.release()

            # ---- avT allocated before eT (it outlives eT; LIFO) ----
            avT_pool = tc.alloc_tile_pool(name="avT", bufs=1)
            avT = avT_pool.tile([128, 8, OWN], F32R)

            # ---- E: eT = mask * exp(scale * kT^T q), bf16 [k, q-own] ----
            eT_pool = tc.alloc_tile_pool(name="eT", bufs=1)
            eT = eT_pool.tile([128, 16, OWN], BF16)
            tmpE = tc.alloc_tile_pool(name="tmpE", bufs=3)
            for ch in range(2 if 'E' in phases else 0):
                for kb in range(16):
                    ps = ps_pool.tile([128, 512], F32, tag="ps")
                    for dib in range(8):
                        nc.tensor.matmul(ps[:], lhsT=kT[:, dib, kb * 128:(kb + 1) * 128],
                                         rhs=qT[:, dib, ch * 512:(ch + 1) * 512],
                                         start=(dib == 0), stop=(dib == 7))
                    tmp = tmpE.tile([128, 512], F32, tag="t")
                    nc.scalar.activation(tmp[:], ps[:], AF.Exp, scale=sc_s[:, 0:1])
                    nc.vector.scalar_tensor_tensor(
                        eT[:, kb, ch * 512:(ch + 1) * 512],
                        in0=qidx_s[:, ch * 512:(ch + 1) * 512],
                        scalar=kidx_s[:, kb:kb + 1],
                        in1=tmp[:],
                        op0=AluOpType.is_ge, op1=AluOpType.mult)
            tmpE.release()

            # ---- F: avT = v^T eT ; rowsums -> recip ; spill avT/recip ----
            vin_pool = tc.alloc_tile_pool(name="vin", bufs=1)
            v_sb = vin_pool.tile([128, 16, D], BF16)
            for kb in range(16 if 'F' in phases else 0):
                nc.sync.dma_start(v_sb[:, kb, :], vs3[:, kb, :])
            for ch in range(2 if 'F' in phases else 0):
                for dob in range(8):
                    ps = ps_pool.tile([128, 512], F32, tag="ps")
                    for kb in range(16):
                        nc.tensor.matmul(ps[:],
                                         lhsT=v_sb[:, kb, dob * 128:(dob + 1) * 128],
                                         rhs=eT[:, kb, ch * 512:(ch + 1) * 512],
                                         start=(kb == 0), stop=(kb == 15))
                    o = avT[:, dob, ch * 512:(ch + 1) * 512]
                    nc.vector.tensor_copy(o, ps[:])
            for qb in range(8 if 'F' in phases else 0):
                ps = ps_pool.tile([128, 512], F32, tag="ps")
                for kb in range(16):
                    nc.tensor.matmul(ps[:, 0:1], lhsT=eT[:, kb, qb * 128:(qb + 1) * 128],
                                     rhs=ones_b[:, 0:1],
                                     start=(kb == 0), stop=(kb == 15))
                nc.vector.reciprocal(recip[:, qb:qb + 1], ps[:, 0:1])
            for dob in range(8 if 'F' in phases else 0):
                nc.sync.dma_start(av3[:, dob, :], avT[:, dob, :])
            if 'F' in phases:
                nc.sync.dma_start(rc_spill, recip[:])
            else:
                nc.vector.memset(recip[:], 1.0)
                nc.sync.dma_start(rc_spill, recip[:])
            vin_pool.release()
            eT_pool.release()
            avT_pool.release()
            qT_pool.release()
            kT_pool.release()
            pst_pool.release()
            ps_pool.release()
            cst.release()

        # ================= TC2: proj + LN2 + MLP =================
        with tile.TileContext(nc, pool_alloc_mode="queue", trace_sim=trace_sim) as tc:
            cst = tc.alloc_tile_pool(name="cst2", bufs=1)
            ident = cst.tile([128, 128], F32)
            make_identity(nc, ident[:])
            bp_s = cst.tile([128, D], F32)
            nc.sync.dma_start(bp_s[:], bp.broadcast_to((128, D)))
            b2_s = cst.tile([128, D], F32)
            nc.sync.dma_start(b2_s[:], b2.broadcast_to((128, D)))
            b1T_s = cst.tile([128, 32], F32)
            nc.sync.dma_start(b1T_s[:], b1T)
            eps_s = cst.tile([128, 1], F32)
            nc.vector.memset(eps_s[:], EPS)
            recip = cst.tile([128, 8], F32)
            nc.sync.dma_start(recip[:], rc_spill)

            ps_pool = tc.alloc_tile_pool(name="ps2", bufs=6, space="PSUM")
            pst_pool = tc.alloc_tile_pool(name="pst2", bufs=2, space="PSUM")

            def ln_stats2(pool, src, tag):
                st = pool.tile([128, 2, 6], F32, tag=f"{tag}_st")
                sv = src.rearrange("p (s d) -> p s d", s=2)
                nc.vector.bn_stats(st[:, 0, :], sv[:, 0, :])
                nc.vector.bn_stats(st[:, 1, :], sv[:, 1, :])
                mv = pool.tile([128, 2], F32, tag=f"{tag}_mv")
                nc.vector.bn_aggr(mv[:], st[:])
                rstd = pool.tile([128, 1], F32, tag=f"{tag}_rs")
                nc.scalar.activation(rstd[:], mv[:, 1:2], AF.Sqrt, bias=eps_s[:, 0:1])
                nc.vector.reciprocal(rstd[:], rstd[:])
                return mv, rstd

            # h2T before avT2: h2T outlives it (LIFO)
            h2T_pool = tc.alloc_tile_pool(name="h2T", bufs=1)
            h2T = h2T_pool.tile([128, 8, OWN], F32R)
            avT_pool = tc.alloc_tile_pool(name="avT2", bufs=1)
            avT = avT_pool.tile([128, 8, OWN], F32R)
            for dob in range(8):
                nc.sync.dma_start(avT[:, dob, :], av3[:, dob, :])

            # ---- G: a = recip * (avT^T Wp) + bp ; LN2 ; transpose ----
            wp_pool = tc.alloc_tile_pool(name="wp", bufs=1)
            wpt = wp_pool.tile([128, 8, D], F32R)
            nc.sync.dma_start(wpt[:], wp3.bitcast(F32R))
            ga = tc.alloc_tile_pool(name="ga", bufs=3)
            for tb in range(8 if 'G' in phases else 0):
                a_t = ga.tile([128, D], F32, tag="a")
                for ch in range(2):
                    ps = ps_pool.tile([128, 512], F32, tag="ps")
                    for dob in range(8):
                        nc.tensor.matmul(ps[:], lhsT=avT[:, dob, tb * 128:(tb + 1) * 128],
                                         rhs=wpt[:, dob, ch * 512:(ch + 1) * 512],
                                         start=(dob == 0), stop=(dob == 7))
                    nc.vector.scalar_tensor_tensor(
                        a_t[:, ch * 512:(ch + 1) * 512], in0=ps[:],
                        scalar=recip[:, tb:tb + 1],
                        in1=bp_s[:, ch * 512:(ch + 1) * 512],
                        op0=AluOpType.mult, op1=AluOpType.add)
                if 'L' not in phases:
                    continue
                mv, rstd = ln_stats2(ga, a_t[:], "g")
                h2_t = ga.tile([128, D], F32, tag="h2")
                nc.vector.tensor_scalar(h2_t[:], a_t[:], mv[:, 0:1], rstd[:],
                                        op0=AluOpType.subtract, op1=AluOpType.mult)
                for db in range(8):
                    pt = pst_pool.tile([128, 128], F32, tag="pst")
                    nc.tensor.transpose(pt[:], h2_t[:, db * 128:(db + 1) * 128], ident[:])
                    nc.vector.tensor_copy(h2T[:, db, tb * 128:(tb + 1) * 128], pt[:])
            ga.release()
            wp_pool.release()
            avT_pool.release()

            # ---- H: m1T = gelu(W1^T h2 + b1) [m, tok] ----
            m1_pool = tc.alloc_tile_pool(name="m1", bufs=1)
            m1T = m1_pool.tile([128, 32, OWN], F32R)
            w1s = tc.alloc_tile_pool(name="w1s", bufs=3)
            wt = None
            for mb in range(32 if 'H' in phases else 0):
                if 'W' in phases or mb == 0:
                    wt = w1s.tile([128, 8, 128], F32R, tag="w1")
                    nc.sync.dma_start(wt[:], w13[:, :, mb * 128:(mb + 1) * 128].bitcast(F32R))
                for ch in range(2):
                    ps = ps_pool.tile([128, 512], F32, tag="ps")
                    for dib in range(8):
                        nc.tensor.matmul(ps[:], lhsT=wt[:, dib, :],
                                         rhs=h2T[:, dib, ch * 512:(ch + 1) * 512],
                                         start=(dib == 0), stop=(dib == 7))
                    nc.scalar.activation(m1T[:, mb, ch * 512:(ch + 1) * 512], ps[:],
                                         AF.Gelu_apprx_tanh, bias=b1T_s[:, mb:mb + 1])
            w1s.release()

            # ---- I: out = m1 @ W2 + b2 ----
            w2s = tc.alloc_tile_pool(name="w2s", bufs=1)
            ost = tc.alloc_tile_pool(name="ost", bufs=3)
            wt = None
            for c4 in range(4 if 'I' in phases else 0):
                if 'W' in phases or c4 == 0:
                    wt = w2s.tile([128, 32, 256], F32R, tag="w2")
                    nc.sync.dma_start(wt[:], w23[:, :, c4 * 256:(c4 + 1) * 256].bitcast(F32R))
                for tb in range(8):
                    ps = ps_pool.tile([128, 512], F32, tag="ps")
                    for mb in range(32):
                        nc.tensor.matmul(ps[:, 0:256],
                                         lhsT=m1T[:, mb, tb * 128:(tb + 1) * 128],
                                         rhs=wt[:, mb, :],
                                         start=(mb == 0), stop=(mb == 31))
                    ot = ost.tile([128, 256], F32, tag="o")
                    nc.vector.tensor_tensor(ot[:], ps[:, 0:256],
                                            b2_s[:, c4 * 256:(c4 + 1) * 256],
                                            op=AluOpType.add)
                    nc.sync.dma_start(
                        out[tb * 128:(tb + 1) * 128, c4 * 256:(c4 + 1) * 256], ot[:])
            if 'I' not in phases:
                for tb in range(8):
                    ot = ost.tile([128, D], F32, tag="o2")
                    nc.vector.memset(ot[:], 0.0)
                    nc.sync.dma_start(out[tb * 128:(tb + 1) * 128, :], ot[:])
            ost.release()
            w2s.release()
            m1_pool.release()
            h2T_pool.release()
            pst_pool.release()
            ps_pool.release()
            cst.release()

    _split_waits(nc)
    return nc


def make_in_maps(x, ln1_g, ln1_b, Wqkv, bqkv, Wproj, bproj, ln2_g, ln2_b, W1, b1, W2, b2):
    f = np.float32
    x = np.asarray(x, f)
    Wqkv = np.asarray(Wqkv, f)
    bqkv = np.asarray(bqkv, f)
    g1 = np.asarray(ln1_g, f)
    be1 = np.asarray(ln1_b, f)
    g2 = np.asarray(ln2_g, f)
    be2 = np.asarray(ln2_b, f)
    Wk_r, Wq_r, Wv_r = Wqkv[:, :D], Wqkv[:, D:2 * D], Wqkv[:, 2 * D:]
    wkf = np.ascontiguousarray(g1[:, None] * Wk_r)
    wqf = np.ascontiguousarray(g1[:, None] * Wq_r)
    wvf = np.ascontiguousarray(g1[:, None] * Wv_r)
    bk = be1 @ Wk_r + bqkv[:D]
    bq = be1 @ Wq_r + bqkv[D:2 * D]
    bvv = be1 @ Wv_r + bqkv[2 * D:]
    W1 = np.asarray(W1, f)
    w1f = np.ascontiguousarray(g2[:, None] * W1)
    b1f = be2 @ W1 + np.asarray(b1, f)
    common = {
        "wk": wkf, "wq": wqf, "wv": wvf,
        "wp": np.ascontiguousarray(np.asarray(Wproj, f)),
        "w1": w1f, "w2": np.ascontiguousarray(np.asarray(W2, f)),
        "bkT": np.ascontiguousarray(bk.reshape(8, 128).T.astype(f)),
        "bqT": np.ascontiguousarray(bq.reshape(8, 128).T.astype(f)),
        "bv": bvv.reshape(1, D).astype(f),
        "bp": np.asarray(bproj, f).reshape(1, D),
        "b1T": np.ascontiguousarray(b1f.reshape(32, 128).T.astype(f)),
        "b2": np.asarray(b2, f).reshape(1, D),
    }
    in_maps = []
    for c in range(N_CORES):
        b, half = c // 2, c % 2
        if half == 0:
            perm = np.arange(T)
        else:
            perm = np.concatenate([np.arange(OWN, T), np.arange(OWN)])
        m = dict(common)
        m["xb"] = np.ascontiguousarray(x[b][perm])
        m["qidx"] = np.broadcast_to(
            np.arange(half * OWN, half * OWN + OWN, dtype=f), (128, OWN)).copy()
        m["kidx"] = np.ascontiguousarray(perm.reshape(16, 128).T.astype(f))
        in_maps.append(m)
    return in_maps


_CACHE = {}


def kernel(**inputs) -> np.ndarray:
    from concourse.bass_utils import run_bass_kernel_spmd
    if "nc" not in _CACHE:
        _CACHE["nc"] = build_nc(rep=1)
    nc = _CACHE["nc"]
    in_maps = make_in_maps(**inputs)
    res = run_bass_kernel_spmd(nc, in_maps, core_ids=list(range(N_CORES)))
    full = np.empty((B, T, D), np.float32)
    for c in range(N_CORES):
        b, half = c // 2, c % 2
        full[b, half * OWN:(half + 1) * OWN, :] = res.results[c]["out"]
    return full


# revision 16
# speedup vs baseline: 39.6479x; 14.7262x over previous
"""Trainium2 Bass kernel for a single-head causal transformer block.

B=4, T=2048, D=1024, MLP 4x. 8 NeuronCores, SPMD: core c handles batch c//2,
token half c%2 (1024 own tokens). Each core recomputes k/v for its batch's
full 2048 tokens (cheaper than a collective). All per-core divergence (token
order, causal mask, q indices) is data-driven so every core runs the same NEFF.

Matmuls run in float32r (TF32-like, full PE rate at free-dim>=256); the
softmax weights and v go through bf16 (post-LN2 renormalization keeps the
effect tiny). LN gains/biases are folded into the next matmul host-side.
"""
import sys
sys.path.insert(0, '/opt/trn_rl_repo')

import math
import numpy as np

import concourse.bass as bass
import concourse.tile as tile
from concourse import mybir
from concourse.alu_op_type import AluOpType

B, T, D = 4, 2048, 1024
DM = 4 * D          # mlp hidden
OWN = 1024          # tokens per core
EPS = 1e-5
SCALE = 1.0 / math.sqrt(T)
F32 = mybir.dt.float32
F32R = mybir.dt.float32r
BF16 = mybir.dt.float16  # fp16: f32r-grade mantissa for attention path
AF = mybir.ActivationFunctionType
N_CORES = 8


def _split_waits(nc, max_waits=1):
    """This walrus build allows at most ONE sync-wait per instruction.
    Move extra waits onto same-engine NoOps placed just before."""
    n_added = 0
    for f in nc.m.functions:
        for blk in f.blocks:
            out, dirty = [], False
            for inst in blk.instructions:
                si = inst.sync_info
                if si is not None and len(si.on_wait) > max_waits:
                    waits = list(si.on_wait)
                    extra, keep = waits[:-max_waits], waits[-max_waits:]
                    for k, w in enumerate(extra):
                        out.append(mybir.InstNoOp(
                            name=f"{inst.name}-ws{k}",
                            engine=inst.engine,
                            sync_info=mybir.SyncInfo(on_wait=[w], on_update=[]),
                            bass_nofuse=True,
                        ))
                        n_added += 1
                    inst.sync_info = mybir.SyncInfo(
                        on_wait=keep, on_update=list(si.on_update))
                    dirty = True
                out.append(inst)
            if dirty:
                blk.instructions = out
    return n_added


def build_nc(rep=1, trace_sim=False, phases='ABCDEFGHILW'):
    from concourse.masks import make_identity

    nc = bass.Bass("TRN2", target_bir_lowering=False, debug=False)
    d = lambda name, shape, dt=F32, out=False: nc.dram_tensor(
        name, shape, dt, kind="ExternalOutput" if out else "ExternalInput").ap()

    xb = d("xb", [T, D])                 # reordered tokens (own first)
    wk, wq, wv, wp = (d(n, [D, D]) for n in ("wk", "wq", "wv", "wp"))
    w1 = d("w1", [D, DM])
    w2 = d("w2", [DM, D])
    bkT = d("bkT", [128, 8])             # k-bias, [dout%128, dout//128]
    bqT = d("bqT", [128, 8])
    bv = d("bv", [1, D])
    bp = d("bp", [1, D])
    b1T = d("b1T", [128, 32])
    b2 = d("b2", [1, D])
    qidx = d("qidx", [128, OWN])         # global q index per own-token column
    kidx = d("kidx", [128, 16])          # global kv index per (partition, kblock)
    out = d("out", [OWN, D], out=True)
    v_spill = nc.dram_tensor("v_spill", [T, D], BF16).ap()
    av_spill = nc.dram_tensor("av_spill", [D, OWN], F32R).ap()
    rc_spill = nc.dram_tensor("rc_spill", [128, 8], F32).ap()

    # [ (s p) n -> p s n ] views: row r = s*128+p
    wk3 = wk.rearrange("(s p) n -> p s n", p=128)
    wq3 = wq.rearrange("(s p) n -> p s n", p=128)
    wv3 = wv.rearrange("(s p) n -> p s n", p=128)
    wp3 = wp.rearrange("(s p) n -> p s n", p=128)
    w13 = w1.rearrange("(s p) n -> p s n", p=128)
    w23 = w2.rearrange("(s p) n -> p s n", p=128)
    vs3 = v_spill.rearrange("(s p) n -> p s n", p=128)
    av3 = av_spill.rearrange("(s p) n -> p s n", p=128)

    for r in range(rep):
        # ================= TC1: LN1, q/k/v, attention =================
        with tile.TileContext(nc, pool_alloc_mode="queue", trace_sim=trace_sim) as tc:
            cst = tc.alloc_tile_pool(name="cst", bufs=1)
            ident = cst.tile([128, 128], F32)
            make_identity(nc, ident[:])
            bkT_s = cst.tile([128, 8], F32)
            nc.sync.dma_start(bkT_s[:], bkT)
            bqT_s = cst.tile([128, 8], F32)
            nc.sync.dma_start(bqT_s[:], bqT)
            qidx_s = cst.tile([128, OWN], F32)
            nc.sync.dma_start(qidx_s[:], qidx)
            kidx_s = cst.tile([128, 16], F32)
            nc.sync.dma_start(kidx_s[:], kidx)
            bv_s = cst.tile([128, D], F32)
            nc.sync.dma_start(bv_s[:], bv.broadcast_to((128, D)))
            ones_f = cst.tile([128, 1], F32)
            nc.vector.memset(ones_f[:], 1.0)
            ones_b = cst.tile([128, 1], BF16)
            nc.vector.tensor_copy(ones_b[:], ones_f[:])
            eps_s = cst.tile([128, 1], F32)
            nc.vector.memset(eps_s[:], EPS)
            sc_s = cst.tile([128, 1], F32)
            nc.vector.memset(sc_s[:], SCALE)
            recip = cst.tile([128, 8], F32)

            ps_pool = tc.alloc_tile_pool(name="ps", bufs=6, space="PSUM")
            pst_pool = tc.alloc_tile_pool(name="pst", bufs=2, space="PSUM")

            def ln_stats(pool, src, tag):
                st = pool.tile([128, 2, 6], F32, tag=f"{tag}_st")
                sv = src.rearrange("p (s d) -> p s d", s=2)
                nc.vector.bn_stats(st[:, 0, :], sv[:, 0, :])
                nc.vector.bn_stats(st[:, 1, :], sv[:, 1, :])
                mv = pool.tile([128, 2], F32, tag=f"{tag}_mv")
                nc.vector.bn_aggr(mv[:], st[:])
                rstd = pool.tile([128, 1], F32, tag=f"{tag}_rs")
                nc.scalar.activation(rstd[:], mv[:, 1:2], AF.Sqrt, bias=eps_s[:, 0:1])
                nc.vector.reciprocal(rstd[:], rstd[:])
                return mv, rstd

            kT_pool = tc.alloc_tile_pool(name="kT", bufs=1)
            kT = kT_pool.tile([128, 8, T], F32R)
            qT_pool = tc.alloc_tile_pool(name="qT", bufs=1)
            qT = qT_pool.tile([128, 8, OWN], F32R)

            # ---- merged LN1 + k/q/v projections, per 1024-token group ----
            work = tc.alloc_tile_pool(name="work", bufs=3)
            hTg_pool = tc.alloc_tile_pool(name="hTg", bufs=1)
            for grp in range(2):
                hTg = hTg_pool.tile([128, 8, 1024], F32R, tag="hTg")
                for tbl in range(8):
                    x_t = work.tile([128, D], F32, tag="x", bufs=2)
                    nc.sync.dma_start(
                        x_t[:], xb[(grp * 8 + tbl) * 128:(grp * 8 + tbl + 1) * 128, :])
                    mv, rstd = ln_stats(work, x_t[:], "a")
                    h_t = work.tile([128, D], F32, tag="h", bufs=2)
                    nc.vector.tensor_scalar(h_t[:], x_t[:], mv[:, 0:1], rstd[:],
                                            op0=AluOpType.subtract, op1=AluOpType.mult)
                    for db in range(8):
                        pt = pst_pool.tile([128, 128], F32, tag="pst")
                        nc.tensor.transpose(pt[:], h_t[:, db * 128:(db + 1) * 128],
                                            ident[:])
                        nc.vector.tensor_copy(hTg[:, db, tbl * 128:(tbl + 1) * 128], pt[:])
                # kT columns for this group (weights in 2KB-segment chunks)
                for wch in range(2):
                    wt = work.tile([128, 8, 512], F32R, tag="wk", bufs=2)
                    nc.sync.dma_start(
                        wt[:], wk3[:, :, wch * 512:(wch + 1) * 512].bitcast(F32R))
                    for dsub in range(4):
                        dob = wch * 4 + dsub
                        for ch in range(2):
                            ps = ps_pool.tile([128, 512], F32, tag="ps")
                            for dib in range(8):
                                nc.tensor.matmul(ps[:], lhsT=wt[:, dib, dsub * 128:(dsub + 1) * 128],
                                                 rhs=hTg[:, dib, ch * 512:(ch + 1) * 512],
                                                 start=(dib == 0), stop=(dib == 7))
                            o = kT[:, dob, grp * 1024 + ch * 512:grp * 1024 + (ch + 1) * 512]
                            if ch % 2:
                                nc.scalar.activation(o, ps[:], AF.Identity,
                                                     bias=bkT_s[:, dob:dob + 1])
                            else:
                                nc.vector.tensor_scalar_add(o, ps[:], bkT_s[:, dob:dob + 1])
                # qT (own tokens are always group 0 — same on every core)
                if grp == 0:
                    for wch in range(2):
                        wt = work.tile([128, 8, 512], F32R, tag="wk", bufs=2)
                        nc.sync.dma_start(
                            wt[:], wq3[:, :, wch * 512:(wch + 1) * 512].bitcast(F32R))
                        for dsub in range(4):
                            dob = wch * 4 + dsub
                            for ch in range(2):
                                ps = ps_pool.tile([128, 512], F32, tag="ps")
                                for dib in range(8):
                                    nc.tensor.matmul(ps[:], lhsT=wt[:, dib, dsub * 128:(dsub + 1) * 128],
                                                     rhs=hTg[:, dib, ch * 512:(ch + 1) * 512],
                                                     start=(dib == 0), stop=(dib == 7))
                                o = qT[:, dob, ch * 512:(ch + 1) * 512]
                                if ch % 2:
                                    nc.scalar.activation(o, ps[:], AF.Identity,
                                                         bias=bqT_s[:, dob:dob + 1])
                                else:
                                    nc.vector.tensor_scalar_add(o, ps[:],
                                                                bqT_s[:, dob:dob + 1])
                # v rows for this group -> bf16 spill
                for ch in range(2):
                    wt = work.tile([128, 8, 512], F32R, tag="wv", bufs=1)
                    nc.sync.dma_start(
                        wt[:], wv3[:, :, ch * 512:(ch + 1) * 512].bitcast(F32R))
                    for tbl in range(8):
                        ps = ps_pool.tile([128, 512], F32, tag="ps")
                        for dib in range(8):
                            nc.tensor.matmul(
                                ps[:], lhsT=hTg[:, dib, tbl * 128:(tbl + 1) * 128],
                                rhs=wt[:, dib, :], start=(dib == 0), stop=(dib == 7))
                        vt = work.tile([128, 512], BF16, tag="vt")
                        nc.vector.tensor_tensor(vt[:], ps[:],
                                                bv_s[:, ch * 512:(ch + 1) * 512],
                                                op=AluOpType.add)
                        tb = grp * 8 + tbl
                        nc.sync.dma_start(
                            v_spill[tb * 128:(tb + 1) * 128, ch * 512:(ch + 1) * 512],
                            vt[:])
            hTg_pool.release()
            work.release()

            # ---- avT allocated before eT (it outlives eT; LIFO) ----
            avT_pool = tc.alloc_tile_pool(name="avT", bufs=1)
            avT = avT_pool.tile([128, 8, OWN], F32R)

            # ---- E: eT = mask * exp(scale * kT^T q), bf16 [k, q-own] ----
            eT_pool = tc.alloc_tile_pool(name="eT", bufs=1)
            eT = eT_pool.tile([128, 16, OWN], BF16)
            tmpE = tc.alloc_tile_pool(name="tmpE", bufs=3)
            for ch in range(2 if 'E' in phases else 0):
                for kb in range(16):
                    ps = ps_pool.tile([128, 512], F32, tag="ps")
                    for dib in range(8):
                        nc.tensor.matmul(ps[:], lhsT=kT[:, dib, kb * 128:(kb + 1) * 128],
                                         rhs=qT[:, dib, ch * 512:(ch + 1) * 512],
                                         start=(dib == 0), stop=(dib == 7))
                    tmp = tmpE.tile([128, 512], F32, tag="t")
                    nc.scalar.activation(tmp[:], ps[:], AF.Exp, scale=sc_s[:, 0:1])
                    nc.vector.scalar_tensor_tensor(
                        eT[:, kb, ch * 512:(ch + 1) * 512],
                        in0=qidx_s[:, ch * 512:(ch + 1) * 512],
                        scalar=kidx_s[:, kb:kb + 1],
                        in1=tmp[:],
                        op0=AluOpType.is_ge, op1=AluOpType.mult)
            tmpE.release()

            # ---- F: avT = v^T eT ; rowsums -> recip ; spill avT/recip ----
            vin_pool = tc.alloc_tile_pool(name="vin", bufs=1)
            v_sb = vin_pool.tile([128, 16, D], BF16)
            for kb in range(16 if 'F' in phases else 0):
                nc.sync.dma_start(v_sb[:, kb, :], vs3[:, kb, :])
            for ch in range(2 if 'F' in phases else 0):
                for dob in range(8):
                    ps = ps_pool.tile([128, 512], F32, tag="ps")
                    for kb in range(16):
                        nc.tensor.matmul(ps[:],
                                         lhsT=v_sb[:, kb, dob * 128:(dob + 1) * 128],
                                         rhs=eT[:, kb, ch * 512:(ch + 1) * 512],
                                         start=(kb == 0), stop=(kb == 15))
                    o = avT[:, dob, ch * 512:(ch + 1) * 512]
                    nc.vector.tensor_copy(o, ps[:])
            for qb in range(8 if 'F' in phases else 0):
                ps = ps_pool.tile([128, 512], F32, tag="ps")
                for kb in range(16):
                    nc.tensor.matmul(ps[:, 0:1], lhsT=eT[:, kb, qb * 128:(qb + 1) * 128],
                                     rhs=ones_b[:, 0:1],
                                     start=(kb == 0), stop=(kb == 15))
                nc.vector.reciprocal(recip[:, qb:qb + 1], ps[:, 0:1])
            for dob in range(8 if 'F' in phases else 0):
                nc.sync.dma_start(av3[:, dob, :], avT[:, dob, :])
            if 'F' in phases:
                nc.sync.dma_start(rc_spill, recip[:])
            else:
                nc.vector.memset(recip[:], 1.0)
                nc.sync.dma_start(rc_spill, recip[:])
            vin_pool.release()
            eT_pool.release()
            avT_pool.release()
            qT_pool.release()
            kT_pool.release()
            pst_pool.release()
            ps_pool.release()
            cst.release()

        # ================= TC2: proj + LN2 + MLP =================
        with tile.TileContext(nc, pool_alloc_mode="queue", trace_sim=trace_sim) as tc:
            cst = tc.alloc_tile_pool(name="cst2", bufs=1)
            ident = cst.tile([128, 128], F32)
            make_identity(nc, ident[:])
            bp_s = cst.tile([128, D], F32)
            nc.sync.dma_start(bp_s[:], bp.broadcast_to((128, D)))
            b2_s = cst.tile([128, D], F32)
            nc.sync.dma_start(b2_s[:], b2.broadcast_to((128, D)))
            b1T_s = cst.tile([128, 32], F32)
            nc.sync.dma_start(b1T_s[:], b1T)
            eps_s = cst.tile([128, 1], F32)
            nc.vector.memset(eps_s[:], EPS)
            recip = cst.tile([128, 8], F32)
            nc.sync.dma_start(recip[:], rc_spill)

            ps_pool = tc.alloc_tile_pool(name="ps2", bufs=6, space="PSUM")
            pst_pool = tc.alloc_tile_pool(name="pst2", bufs=2, space="PSUM")

            def ln_stats2(pool, src, tag):
                st = pool.tile([128, 2, 6], F32, tag=f"{tag}_st")
                sv = src.rearrange("p (s d) -> p s d", s=2)
                nc.vector.bn_stats(st[:, 0, :], sv[:, 0, :])
                nc.vector.bn_stats(st[:, 1, :], sv[:, 1, :])
                mv = pool.tile([128, 2], F32, tag=f"{tag}_mv")
                nc.vector.bn_aggr(mv[:], st[:])
                rstd = pool.tile([128, 1], F32, tag=f"{tag}_rs")
                nc.scalar.activation(rstd[:], mv[:, 1:2], AF.Sqrt, bias=eps_s[:, 0:1])
                nc.vector.reciprocal(rstd[:], rstd[:])
                return mv, rstd

            # h2T before avT2: h2T outlives it (LIFO)
            h2T_pool = tc.alloc_tile_pool(name="h2T", bufs=1)
            h2T = h2T_pool.tile([128, 8, OWN], F32R)
            avT_pool = tc.alloc_tile_pool(name="avT2", bufs=1)
            avT = avT_pool.tile([128, 8, OWN], F32R)
            for dob in range(8):
                nc.sync.dma_start(avT[:, dob, :], av3[:, dob, :])

            # ---- G: a = recip * (avT^T Wp) + bp ; LN2 ; transpose ----
            wp_pool = tc.alloc_tile_pool(name="wp", bufs=1)
            wpt = wp_pool.tile([128, 8, D], F32R)
            nc.sync.dma_start(wpt[:], wp3.bitcast(F32R))
            ga = tc.alloc_tile_pool(name="ga", bufs=3)
            for tb in range(8 if 'G' in phases else 0):
                a_t = ga.tile([128, D], F32, tag="a")
                for ch in range(2):
                    ps = ps_pool.tile([128, 512], F32, tag="ps")
                    for dob in range(8):
                        nc.tensor.matmul(ps[:], lhsT=avT[:, dob, tb * 128:(tb + 1) * 128],
                                         rhs=wpt[:, dob, ch * 512:(ch + 1) * 512],
                                         start=(dob == 0), stop=(dob == 7))
                    nc.vector.scalar_tensor_tensor(
                        a_t[:, ch * 512:(ch + 1) * 512], in0=ps[:],
                        scalar=recip[:, tb:tb + 1],
                        in1=bp_s[:, ch * 512:(ch + 1) * 512],
                        op0=AluOpType.mult, op1=AluOpType.add)
                if 'L' not in phases:
                    continue
                mv, rstd = ln_stats2(ga, a_t[:], "g")
                h2_t = ga.tile([128, D], F32, tag="h2")
                nc.vector.tensor_scalar(h2_t[:], a_t[:], mv[:, 0:1], rstd[:],
                                        op0=AluOpType.subtract, op1=AluOpType.mult)
                for db in range(8):
                    pt = pst_pool.tile([128, 128], F32, tag="pst")
                    nc.tensor.transpose(pt[:], h2_t[:, db * 128:(db + 1) * 128], ident[:])
                    nc.vector.tensor_copy(h2T[:, db, tb * 128:(tb + 1) * 128], pt[:])
            ga.release()
            wp_pool.release()
            avT_pool.release()

            # ---- H: m1T = gelu(W1^T h2 + b1) [m, tok] ----
            m1_pool = tc.alloc_tile_pool(name="m1", bufs=1)
            m1T = m1_pool.tile([128, 32, OWN], F32R)
            w1s = tc.alloc_tile_pool(name="w1s", bufs=3)
            for wch in range(8 if 'H' in phases else 0):
                wt = w1s.tile([128, 8, 512], F32R, tag="w1", bufs=2)
                nc.sync.dma_start(wt[:], w13[:, :, wch * 512:(wch + 1) * 512].bitcast(F32R))
                for msub in range(4):
                    mb = wch * 4 + msub
                    for ch in range(2):
                        ps = ps_pool.tile([128, 512], F32, tag="ps")
                        for dib in range(8):
                            nc.tensor.matmul(ps[:], lhsT=wt[:, dib, msub * 128:(msub + 1) * 128],
                                             rhs=h2T[:, dib, ch * 512:(ch + 1) * 512],
                                             start=(dib == 0), stop=(dib == 7))
                        nc.scalar.activation(m1T[:, mb, ch * 512:(ch + 1) * 512], ps[:],
                                             AF.Gelu_apprx_tanh, bias=b1T_s[:, mb:mb + 1])
            w1s.release()

            # ---- I: out = m1 @ W2 + b2 ----
            w2s = tc.alloc_tile_pool(name="w2s", bufs=1)
            ost = tc.alloc_tile_pool(name="ost", bufs=3)
            wt = None
            for c4 in range(4 if 'I' in phases else 0):
                if 'W' in phases or c4 == 0:
                    wt = w2s.tile([128, 32, 256], F32R, tag="w2")
                    nc.sync.dma_start(wt[:], w23[:, :, c4 * 256:(c4 + 1) * 256].bitcast(F32R))
                for tb in range(8):
                    ps = ps_pool.tile([128, 512], F32, tag="ps")
                    for mb in range(32):
                        nc.tensor.matmul(ps[:, 0:256],
                                         lhsT=m1T[:, mb, tb * 128:(tb + 1) * 128],
                                         rhs=wt[:, mb, :],
                                         start=(mb == 0), stop=(mb == 31))
                    ot = ost.tile([128, 256], F32, tag="o")
                    nc.vector.tensor_tensor(ot[:], ps[:, 0:256],
                                            b2_s[:, c4 * 256:(c4 + 1) * 256],
                                            op=AluOpType.add)
                    nc.sync.dma_start(
                        out[tb * 128:(tb + 1) * 128, c4 * 256:(c4 + 1) * 256], ot[:])
            if 'I' not in phases:
                for tb in range(8):
                    ot = ost.tile([128, D], F32, tag="o2")
                    nc.vector.memset(ot[:], 0.0)
                    nc.sync.dma_start(out[tb * 128:(tb + 1) * 128, :], ot[:])
            ost.release()
            w2s.release()
            m1_pool.release()
            h2T_pool.release()
            pst_pool.release()
            ps_pool.release()
            cst.release()

    _split_waits(nc)
    return nc


def make_in_maps(x, ln1_g, ln1_b, Wqkv, bqkv, Wproj, bproj, ln2_g, ln2_b, W1, b1, W2, b2):
    f = np.float32
    x = np.asarray(x, f)
    Wqkv = np.asarray(Wqkv, f)
    bqkv = np.asarray(bqkv, f)
    g1 = np.asarray(ln1_g, f)
    be1 = np.asarray(ln1_b, f)
    g2 = np.asarray(ln2_g, f)
    be2 = np.asarray(ln2_b, f)
    Wk_r, Wq_r, Wv_r = Wqkv[:, :D], Wqkv[:, D:2 * D], Wqkv[:, 2 * D:]
    wkf = np.ascontiguousarray(g1[:, None] * Wk_r)
    wqf = np.ascontiguousarray(g1[:, None] * Wq_r)
    wvf = np.ascontiguousarray(g1[:, None] * Wv_r)
    bk = be1 @ Wk_r + bqkv[:D]
    bq = be1 @ Wq_r + bqkv[D:2 * D]
    bvv = be1 @ Wv_r + bqkv[2 * D:]
    W1 = np.asarray(W1, f)
    w1f = np.ascontiguousarray(g2[:, None] * W1)
    b1f = be2 @ W1 + np.asarray(b1, f)
    common = {
        "wk": wkf, "wq": wqf, "wv": wvf,
        "wp": np.ascontiguousarray(np.asarray(Wproj, f)),
        "w1": w1f, "w2": np.ascontiguousarray(np.asarray(W2, f)),
        "bkT": np.ascontiguousarray(bk.reshape(8, 128).T.astype(f)),
        "bqT": np.ascontiguousarray(bq.reshape(8, 128).T.astype(f)),
        "bv": bvv.reshape(1, D).astype(f),
        "bp": np.asarray(bproj, f).reshape(1, D),
        "b1T": np.ascontiguousarray(b1f.reshape(32, 128).T.astype(f)),
        "b2": np.asarray(b2, f).reshape(1, D),
    }
    in_maps = []
    for c in range(N_CORES):
        b, half = c // 2, c % 2
        if half == 0:
            perm = np.arange(T)
        else:
            perm = np.concatenate([np.arange(OWN, T), np.arange(OWN)])
        m = dict(common)
        m["xb"] = np.ascontiguousarray(x[b][perm])
        m["qidx"] = np.broadcast_to(
            np.arange(half * OWN, half * OWN + OWN, dtype=f), (128, OWN)).copy()
        m["kidx"] = np.ascontiguousarray(perm.reshape(16, 128).T.astype(f))
        in_maps.append(m)
    return in_maps


_CACHE = {}


def kernel(**inputs) -> np.ndarray:
    from concourse.bass_utils import run_bass_kernel_spmd
    if "nc" not in _CACHE:
        _CACHE["nc"] = build_nc(rep=1)
    nc = _CACHE["nc"]
    in_maps = make_in_maps(**inputs)
    res = run_bass_kernel_spmd(nc, in_maps, core_ids=list(range(N_CORES)))
    full = np.empty((B, T, D), np.float32)
    for c in range(N_CORES):
        b, half = c // 2, c % 2
        full[b, half * OWN:(half + 1) * OWN, :] = res.results[c]["out"]
    return full
